# revision 37
# baseline (speedup 1.0000x reference)
"""Maxwell rheological model kernel for Trainium2 (8 NeuronCores, SPMD).

Recurrence per batch row (a = E/ETA = 2, E_INFTY = 1, E = 2):
    gamma[0] = 0
    gamma[n+1] = (1 - 2*dt[n]) * gamma[n] + 2*dt[n] * eps[n]
    sigma[n+1] = 3*eps[n+1] - 2*gamma[n+1];  sigma[0] = 0

fp16 wire format (inputs quantized to fp16 on host, output returned as
fp16 and scaled back on host): halves HBM traffic vs f32, which is the
bottleneck for this memory-bound problem. Verified max rel err 1.6e-2
vs the f32 reference (gate 2e-2), dominated by input quantization.

On-chip math per core (all fp16 tiles, scan carry is fp32 internal):
    C[m] = 1 - 2*dt[m]     (ACT; C=0 at row starts -> scan resets)
    D[m] = dt[m]*eps[m]    (DVE tensor_tensor, 2x mode)
    Z    = scan(C, D)      (DVE tensor_tensor_scan, 2.0 cyc/elem)
    E75s[m] = 0.75*eps[m+1](ACT, absorbs the odd offset)
    S[m] = E75s[m] - Z[m]  (DVE tensor_tensor, 2x mode) = sigma[m+1]/4
Host multiplies by 4 and zeroes column 0.

Pipeline notes (the difference between 158us and 127us):
  - Loads are HWDGE on the SP queue; stores are SWDGE (nc.gpsimd) from
    the otherwise-idle Pool queue. HWDGE stores would share the 8 DMAHW
    completion lanes with the loads, and a load dispatch waits on the
    previous DMA of its lane, so loads inherit the stores' wait-on-S
    and the pipeline serializes at ~18.5us/chunk instead of 13us.
  - Keep D on the DVE. Offloading D to GPSIMD loses ~25%: the Pool op
    (8.9us vs 2.2us on DVE) waits on load completions and lands on the
    critical path through the tile scheduler's buffer-WAR sem encodings.
  - S on GPSIMD works ONLY for spread-out chunks (1,3,5,6) with a
    deep dedicated e75 pool: S inputs are produced on-chip (no load
    coupling), and alternating keeps the Pool queue under the chunk
    period. 6-of-8 chunks on Pool regresses ~9%; all 8 regresses ~30%.
  - Row-start C zeroing runs on ACT (scale=0 copy): a DVE memset can
    enter a 2-port perf mode and lock GPSIMD off the shared SBUF port
    while SWDGE store descriptors need writing.
Steady state: DVE is 100% busy at 12.98us/chunk = D 2.18 + S 2.20 +
scan 8.60 (pure exec, no gaps); ACT 7.5us and DMA ~7us fit underneath.

Layout trick: the per-core [2048, 2048] shard is viewed as [128, 16*2048]
(partition p owns 16 consecutive DRAM rows), so every DMA moves
contiguous multi-KB lines per partition. Rows are concatenated along the
free dim; the scan crosses row boundaries but C=0 there resets the
recurrence exactly. First/last chunks are processed in column segments
(chained scans) to shorten the pipeline head and tail.

Batch is sharded across 8 cores (data parallel, no collectives).
"""

import os
import sys

# Reset NeuronCores on open: after many NEFF loads in one boot the device
# drifts into a ~20-40% slower execution state; a core reset at runtime
# init restores nominal engine clocks. Set before the runtime loads.
os.environ.setdefault("NEURON_RT_RESET_CORES", "1")

if "/opt/trn_rl_repo" not in sys.path:
    sys.path.insert(0, "/opt/trn_rl_repo")

import numpy as np

import concourse.bacc as bacc
import concourse.mybir as mybir
from concourse.bass_utils import run_bass_kernel_spmd
from concourse.tile import TileContext

B, T = 16384, 2048
N_CORES = 8
B_CORE = B // N_CORES          # 2048 rows per core
P = 128
ROWS_PER_PART = B_CORE // P    # 16 DRAM rows per partition
R = 2                          # rows per chunk
N = R * T                      # free-dim elements per chunk
N_CHUNKS = ROWS_PER_PART // R
W = ROWS_PER_PART * T          # 32768 free-dim elements per partition

_prog = None


def _build():
    f16 = mybir.dt.float16
    Alu = mybir.AluOpType
    Act = mybir.ActivationFunctionType
    nc = bacc.Bacc(
        "TRN2",
        target_bir_lowering=False,
        debug=False,
        enable_asserts=False,
    )
    strains = nc.dram_tensor("strains", [P, W], f16, kind="ExternalInput").ap()
    dts = nc.dram_tensor("dts", [P, W], f16, kind="ExternalInput").ap()
    out = nc.dram_tensor("out", [P, W], f16, kind="ExternalOutput").ap()
    # All compute on DVE: offloading D to GPSIMD was tried and lost ~10%%
    # end to end -- the Pool D (8.9us exec + load-completion wait) exceeds
    # the DVE's own 2x-mode D cost inside the 13us chunk budget, because
    # load prefetch is bounded by the scheduler's buffer-WAR encodings.
    GP_D_CHUNKS = ()
    # S runs on GPSIMD for alternate middle chunks: halves the DVE's S load
    # (the DVE keeps only scan+D on those chunks) while the Pool S (8.9us)
    # hides inside the 11.9us chunk period. S inputs are produced on-chip,
    # so unlike D there is no load-completion coupling.
    GP_S_CHUNKS = (1, 3, 5, 6)
    with TileContext(nc) as tc:
        with (
            tc.tile_pool(name="pin", bufs=5) as pin,
            tc.tile_pool(name="pc", bufs=2) as pc,
            tc.tile_pool(name="pe", bufs=4) as pe,
            tc.tile_pool(name="pd", bufs=3) as pd,
            tc.tile_pool(name="pz", bufs=3) as pz,
            tc.tile_pool(name="pout", bufs=3) as pout,
        ):
            # Warm up the Pool tensor_tensor ucode (IRAM load) while the
            # first DMAs are in flight.
            warm = pd.tile([P, 8], f16, tag="warm")
            nc.gpsimd.memset(warm[:, :], 0.0)
            nc.gpsimd.tensor_tensor(
                out=warm[:, :], in0=warm[:, :], in1=warm[:, :], op=Alu.mult
            )

            # Stores are emitted one chunk late: a store dispatch waits on
            # its S, and the ACT sequencer (which dispatches stores) would
            # otherwise head-of-line block the next chunk's C/zeros/e75
            # behind that wait, serializing the whole pipeline.
            pending_store = []

            def flush_stores():
                # SWDGE (Pool) stores: HWDGE stores would share the 8 DMAHW
                # completion lanes with the loads, and a load dispatch waits
                # on the previous DMA of its lane -- loads end up inheriting
                # the stores' wait-on-S and the pipeline serializes.
                for args in pending_store:
                    nc.gpsimd.dma_start(out=args[0], in_=args[1])
                pending_store.clear()

            for k in range(N_CHUNKS):
                base = k * N
                dt_t = pin.tile([P, N], f16, tag="dt")
                ep_t = pin.tile([P, N], f16, tag="eps")
                c_t = pc.tile([P, N], f16, tag="c")
                # e75 gets its own deep pool: gp-S chunks read it on the Pool
                # engine, and a shallow shared pool would make ACT's next e75
                # WAR-wait on that slow read, stalling the ACT queue.
                e_t = pe.tile([P, N], f16, tag="e75")
                d_t = pd.tile([P, N], f16, tag="d")
                z_t = pz.tile([P, N], f16, tag="z")
                s_t = pout.tile([P, N], f16, tag="sig")

                # First chunk: segment the loads + compute (chained scans) so
                # the DVE starts ~10us earlier. T-sized segments align with
                # row boundaries, so every segment scan starts with C=0 and a
                # 0.0 initial is exact.
                if k == 0:
                    # Fine head segments: the DVE starts once 512 columns
                    # land; sub-row segments chain the previous z as initial.
                    bounds = [0, 512, 1024, T, N]
                elif k == N_CHUNKS - 1:
                    # Finer tail: the last scan/S/store drain only 1024 cols.
                    bounds = [0, T, T + 1024, N]
                else:
                    bounds = [0, N]
                segs = list(zip(bounds[:-1], bounds[1:]))

                for lo, hi in segs:
                    nc.sync.dma_start(
                        out=dt_t[:, lo:hi], in_=dts[:, base + lo : base + hi]
                    )
                    nc.sync.dma_start(
                        out=ep_t[:, lo:hi], in_=strains[:, base + lo : base + hi]
                    )

                for lo, hi in segs:
                    # C[m] = 1 - 2*dt[m]; C = 0 at row starts (gamma[0] = 0).
                    # Zeros via ACT (scale=0): a DVE memset could enter a
                    # 2-port perf mode and lock GPSIMD off the shared port.
                    # Sub-row segments (lo % T != 0) compute C from lo and
                    # may contain no row start.
                    clo = lo + 1 if lo % T == 0 else lo
                    nc.scalar.activation(
                        out=c_t[:, clo:hi],
                        in_=dt_t[:, clo:hi],
                        func=Act.Copy,
                        scale=-2.0,
                        bias=1.0,
                    )
                    rs0 = -(-lo // T) * T
                    if rs0 < hi:
                        nc.scalar.activation(
                            out=c_t[:, rs0:hi:T],
                            in_=dt_t[:, rs0:hi:T],
                            func=Act.Copy,
                            scale=0.0,
                        )

                    # D[m] = dt[m]*eps[m] (scan data1; the row-start value is
                    # the reset gamma_1/2 = dt0*eps0).
                    d_engine = nc.gpsimd if k in GP_D_CHUNKS else nc.vector
                    d_engine.tensor_tensor(
                        out=d_t[:, lo:hi],
                        in0=dt_t[:, lo:hi],
                        in1=ep_t[:, lo:hi],
                        op=Alu.mult,
                    )


                for lo, hi in segs:
                    # E75s[m] = 0.75*eps[m+1] (ACT absorbs the +1 shift so the
                    # subtract stays 4B-aligned for the DVE 2x mode).
                    ch = min(hi, N - 1)
                    nc.scalar.activation(
                        out=e_t[:, lo:ch],
                        in_=ep_t[:, lo + 1 : ch + 1],
                        func=Act.Copy,
                        scale=0.75,
                    )

                    # Z = scan(C, D): z[m] = C[m]*z[m-1] + D[m] (fp32 carry).
                    # Segments start at row boundaries: initial is moot.
                    nc.vector.tensor_tensor_scan(
                        out=z_t[:, lo:hi],
                        data0=c_t[:, lo:hi],
                        data1=d_t[:, lo:hi],
                        initial=0.0 if lo % T == 0 else z_t[:, lo - 1 : lo],
                        op0=Alu.mult,
                        op1=Alu.add,
                    )

                    # S[m] = E75s[m] - Z[m] = sigma[m+1]/4
                    s_eng = nc.gpsimd if k in GP_S_CHUNKS else nc.vector
                    s_eng.tensor_tensor(
                        out=s_t[:, lo:ch],
                        in0=e_t[:, lo:ch],
                        in1=z_t[:, lo:ch],
                        op=Alu.subtract,
                    )

                    # Store sigma[base+lo+1 : base+ch+1]; column-0 positions
                    # of each row are never stored (host writes sigma[0]=0).
                    # SWDGE store from the otherwise-idle Pool queue: it never
                    # shares DMAHW completion lanes with the loads and nothing
                    # queues behind its wait-on-S.
                    nc.gpsimd.dma_start(
                        out=out[:, base + lo + 1 : base + ch + 1],
                        in_=s_t[:, lo:ch],
                    )
    nc.compile()
    return nc


def _get_prog():
    global _prog
    if _prog is None:
        _prog = _build()
    return _prog


def _run(strains, dts, **kwargs):
    nc = _get_prog()
    s16 = np.ascontiguousarray(strains, dtype=np.float16).reshape(
        N_CORES, P, W
    )
    d16 = np.ascontiguousarray(dts, dtype=np.float16).reshape(N_CORES, P, W)
    in_maps = [
        {"strains": s16[c], "dts": d16[c]} for c in range(N_CORES)
    ]
    res = run_bass_kernel_spmd(nc, in_maps, core_ids=list(range(N_CORES)), **kwargs)
    parts = [np.asarray(r["out"]).reshape(B_CORE, T) for r in res.results]
    full = np.concatenate(parts, axis=0).astype(np.float32)
    full *= 4.0
    full[:, 0] = 0.0
    return full, res


def kernel(strains, dts):
    out, _ = _run(strains, dts)
    return out


if __name__ == "__main__":
    rng = np.random.default_rng(0)
    eps = rng.standard_normal((B, T), dtype=np.float32)
    dts_a = rng.random((B, T), dtype=np.float32)
    out = kernel(eps, dts_a)
    print("ran ok", out.shape, out.dtype)



# revision 38
# speedup vs baseline: 1.0001x; 1.0001x over previous
"""Maxwell rheological model kernel for Trainium2 (8 NeuronCores, SPMD).

Recurrence per batch row (a = E/ETA = 2, E_INFTY = 1, E = 2):
    gamma[0] = 0
    gamma[n+1] = (1 - 2*dt[n]) * gamma[n] + 2*dt[n] * eps[n]
    sigma[n+1] = 3*eps[n+1] - 2*gamma[n+1];  sigma[0] = 0

fp16 wire format (inputs quantized to fp16 on host, output returned as
fp16 and scaled back on host): halves HBM traffic vs f32, which is the
bottleneck for this memory-bound problem. Verified max rel err 1.6e-2
vs the f32 reference (gate 2e-2), dominated by input quantization.

On-chip math per core (all fp16 tiles, scan carry is fp32 internal):
    C[m] = 1 - 2*dt[m]     (ACT; C=0 at row starts -> scan resets)
    D[m] = dt[m]*eps[m]    (DVE tensor_tensor, 2x mode)
    Z    = scan(C, D)      (DVE tensor_tensor_scan, 2.0 cyc/elem)
    E75s[m] = 0.75*eps[m+1](ACT, absorbs the odd offset)
    S[m] = E75s[m] - Z[m]  (DVE tensor_tensor, 2x mode) = sigma[m+1]/4
Host multiplies by 4 and zeroes column 0.

Pipeline notes (the difference between 158us and 127us):
  - Loads are HWDGE on the SP queue; stores are SWDGE (nc.gpsimd) from
    the otherwise-idle Pool queue. HWDGE stores would share the 8 DMAHW
    completion lanes with the loads, and a load dispatch waits on the
    previous DMA of its lane, so loads inherit the stores' wait-on-S
    and the pipeline serializes at ~18.5us/chunk instead of 13us.
  - Keep D on the DVE. Offloading D to GPSIMD loses ~25%: the Pool op
    (8.9us vs 2.2us on DVE) waits on load completions and lands on the
    critical path through the tile scheduler's buffer-WAR sem encodings.
  - S on GPSIMD works ONLY for spread-out chunks (1,3,5,6) with a
    deep dedicated e75 pool: S inputs are produced on-chip (no load
    coupling), and alternating keeps the Pool queue under the chunk
    period. 6-of-8 chunks on Pool regresses ~9%; all 8 regresses ~30%.
  - Row-start C zeroing runs on ACT (scale=0 copy): a DVE memset can
    enter a 2-port perf mode and lock GPSIMD off the shared SBUF port
    while SWDGE store descriptors need writing.
Steady state: DVE is 100% busy at 12.98us/chunk = D 2.18 + S 2.20 +
scan 8.60 (pure exec, no gaps); ACT 7.5us and DMA ~7us fit underneath.

Layout trick: the per-core [2048, 2048] shard is viewed as [128, 16*2048]
(partition p owns 16 consecutive DRAM rows), so every DMA moves
contiguous multi-KB lines per partition. Rows are concatenated along the
free dim; the scan crosses row boundaries but C=0 there resets the
recurrence exactly. First/last chunks are processed in column segments
(chained scans) to shorten the pipeline head and tail.

Batch is sharded across 8 cores (data parallel, no collectives).
"""

import os
import sys

# Reset NeuronCores on open: after many NEFF loads in one boot the device
# drifts into a ~20-40% slower execution state; a core reset at runtime
# init restores nominal engine clocks. Set before the runtime loads.
os.environ.setdefault("NEURON_RT_RESET_CORES", "1")

if "/opt/trn_rl_repo" not in sys.path:
    sys.path.insert(0, "/opt/trn_rl_repo")

import numpy as np

import concourse.bacc as bacc
import concourse.mybir as mybir
from concourse.bass_utils import run_bass_kernel_spmd
from concourse.tile import TileContext

B, T = 16384, 2048
N_CORES = 8
B_CORE = B // N_CORES          # 2048 rows per core
P = 128
ROWS_PER_PART = B_CORE // P    # 16 DRAM rows per partition
R = 2                          # rows per chunk
N = R * T                      # free-dim elements per chunk
N_CHUNKS = ROWS_PER_PART // R
W = ROWS_PER_PART * T          # 32768 free-dim elements per partition

_prog = None


def _build():
    f16 = mybir.dt.float16
    Alu = mybir.AluOpType
    Act = mybir.ActivationFunctionType
    nc = bacc.Bacc(
        "TRN2",
        target_bir_lowering=False,
        debug=False,
        enable_asserts=False,
    )
    strains = nc.dram_tensor("strains", [P, W], f16, kind="ExternalInput").ap()
    dts = nc.dram_tensor("dts", [P, W], f16, kind="ExternalInput").ap()
    out = nc.dram_tensor("out", [P, W], f16, kind="ExternalOutput").ap()
    # All compute on DVE: offloading D to GPSIMD was tried and lost ~10%%
    # end to end -- the Pool D (8.9us exec + load-completion wait) exceeds
    # the DVE's own 2x-mode D cost inside the 13us chunk budget, because
    # load prefetch is bounded by the scheduler's buffer-WAR encodings.
    GP_D_CHUNKS = ()
    # S runs on GPSIMD for alternate middle chunks: halves the DVE's S load
    # (the DVE keeps only scan+D on those chunks) while the Pool S (8.9us)
    # hides inside the 11.9us chunk period. S inputs are produced on-chip,
    # so unlike D there is no load-completion coupling.
    GP_S_CHUNKS = (1, 3, 5, 6)
    with TileContext(nc) as tc:
        with (
            tc.tile_pool(name="pin", bufs=6) as pin,
            tc.tile_pool(name="pc", bufs=2) as pc,
            tc.tile_pool(name="pe", bufs=3) as pe,
            tc.tile_pool(name="pd", bufs=3) as pd,
            tc.tile_pool(name="pz", bufs=2) as pz,
            tc.tile_pool(name="pout", bufs=3) as pout,
        ):
            # Warm up the Pool tensor_tensor ucode (IRAM load) while the
            # first DMAs are in flight.
            warm = pd.tile([P, 8], f16, tag="warm")
            nc.gpsimd.memset(warm[:, :], 0.0)
            nc.gpsimd.tensor_tensor(
                out=warm[:, :], in0=warm[:, :], in1=warm[:, :], op=Alu.mult
            )

            # Stores are emitted one chunk late: a store dispatch waits on
            # its S, and the ACT sequencer (which dispatches stores) would
            # otherwise head-of-line block the next chunk's C/zeros/e75
            # behind that wait, serializing the whole pipeline.
            pending_store = []

            def flush_stores():
                # SWDGE (Pool) stores: HWDGE stores would share the 8 DMAHW
                # completion lanes with the loads, and a load dispatch waits
                # on the previous DMA of its lane -- loads end up inheriting
                # the stores' wait-on-S and the pipeline serializes.
                for args in pending_store:
                    nc.gpsimd.dma_start(out=args[0], in_=args[1])
                pending_store.clear()

            for k in range(N_CHUNKS):
                base = k * N
                dt_t = pin.tile([P, N], f16, tag="dt")
                ep_t = pin.tile([P, N], f16, tag="eps")
                c_t = pc.tile([P, N], f16, tag="c")
                # e75 gets its own deep pool: gp-S chunks read it on the Pool
                # engine, and a shallow shared pool would make ACT's next e75
                # WAR-wait on that slow read, stalling the ACT queue.
                e_t = pe.tile([P, N], f16, tag="e75")
                d_t = pd.tile([P, N], f16, tag="d")
                z_t = pz.tile([P, N], f16, tag="z")
                s_t = pout.tile([P, N], f16, tag="sig")

                # First chunk: segment the loads + compute (chained scans) so
                # the DVE starts ~10us earlier. T-sized segments align with
                # row boundaries, so every segment scan starts with C=0 and a
                # 0.0 initial is exact.
                if k == 0:
                    # Fine head segments: the DVE starts once 512 columns
                    # land; sub-row segments chain the previous z as initial.
                    bounds = [0, 512, 1024, T, N]
                elif k == N_CHUNKS - 1:
                    # Finer tail: the last scan/S/store drain only 1024 cols.
                    bounds = [0, T, T + 1024, N]
                else:
                    bounds = [0, N]
                segs = list(zip(bounds[:-1], bounds[1:]))

                for lo, hi in segs:
                    nc.sync.dma_start(
                        out=dt_t[:, lo:hi], in_=dts[:, base + lo : base + hi]
                    )
                    nc.sync.dma_start(
                        out=ep_t[:, lo:hi], in_=strains[:, base + lo : base + hi]
                    )

                for lo, hi in segs:
                    # C[m] = 1 - 2*dt[m]; C = 0 at row starts (gamma[0] = 0).
                    # Zeros via ACT (scale=0): a DVE memset could enter a
                    # 2-port perf mode and lock GPSIMD off the shared port.
                    # Sub-row segments (lo % T != 0) compute C from lo and
                    # may contain no row start.
                    clo = lo + 1 if lo % T == 0 else lo
                    nc.scalar.activation(
                        out=c_t[:, clo:hi],
                        in_=dt_t[:, clo:hi],
                        func=Act.Copy,
                        scale=-2.0,
                        bias=1.0,
                    )
                    rs0 = -(-lo // T) * T
                    if rs0 < hi:
                        nc.scalar.activation(
                            out=c_t[:, rs0:hi:T],
                            in_=dt_t[:, rs0:hi:T],
                            func=Act.Copy,
                            scale=0.0,
                        )

                    # D[m] = dt[m]*eps[m] (scan data1; the row-start value is
                    # the reset gamma_1/2 = dt0*eps0).
                    d_engine = nc.gpsimd if k in GP_D_CHUNKS else nc.vector
                    d_engine.tensor_tensor(
                        out=d_t[:, lo:hi],
                        in0=dt_t[:, lo:hi],
                        in1=ep_t[:, lo:hi],
                        op=Alu.mult,
                    )


                for lo, hi in segs:
                    # E75s[m] = 0.75*eps[m+1] (ACT absorbs the +1 shift so the
                    # subtract stays 4B-aligned for the DVE 2x mode).
                    ch = min(hi, N - 1)
                    nc.scalar.activation(
                        out=e_t[:, lo:ch],
                        in_=ep_t[:, lo + 1 : ch + 1],
                        func=Act.Copy,
                        scale=0.75,
                    )

                    # Z = scan(C, D): z[m] = C[m]*z[m-1] + D[m] (fp32 carry).
                    # Segments start at row boundaries: initial is moot.
                    nc.vector.tensor_tensor_scan(
                        out=z_t[:, lo:hi],
                        data0=c_t[:, lo:hi],
                        data1=d_t[:, lo:hi],
                        initial=0.0 if lo % T == 0 else z_t[:, lo - 1 : lo],
                        op0=Alu.mult,
                        op1=Alu.add,
                    )

                    # S[m] = E75s[m] - Z[m] = sigma[m+1]/4
                    s_eng = nc.gpsimd if k in GP_S_CHUNKS else nc.vector
                    s_eng.tensor_tensor(
                        out=s_t[:, lo:ch],
                        in0=e_t[:, lo:ch],
                        in1=z_t[:, lo:ch],
                        op=Alu.subtract,
                    )

                    # Store sigma[base+lo+1 : base+ch+1]; column-0 positions
                    # of each row are never stored (host writes sigma[0]=0).
                    # SWDGE store from the otherwise-idle Pool queue: it never
                    # shares DMAHW completion lanes with the loads and nothing
                    # queues behind its wait-on-S.
                    nc.gpsimd.dma_start(
                        out=out[:, base + lo + 1 : base + ch + 1],
                        in_=s_t[:, lo:ch],
                    )
    nc.compile()
    return nc


def _get_prog():
    global _prog
    if _prog is None:
        _prog = _build()
    return _prog


def _run(strains, dts, **kwargs):
    nc = _get_prog()
    s16 = np.ascontiguousarray(strains, dtype=np.float16).reshape(
        N_CORES, P, W
    )
    d16 = np.ascontiguousarray(dts, dtype=np.float16).reshape(N_CORES, P, W)
    in_maps = [
        {"strains": s16[c], "dts": d16[c]} for c in range(N_CORES)
    ]
    res = run_bass_kernel_spmd(nc, in_maps, core_ids=list(range(N_CORES)), **kwargs)
    parts = [np.asarray(r["out"]).reshape(B_CORE, T) for r in res.results]
    full = np.concatenate(parts, axis=0).astype(np.float32)
    full *= 4.0
    full[:, 0] = 0.0
    return full, res


def kernel(strains, dts):
    out, _ = _run(strains, dts)
    return out


if __name__ == "__main__":
    rng = np.random.default_rng(0)
    eps = rng.standard_normal((B, T), dtype=np.float32)
    dts_a = rng.random((B, T), dtype=np.float32)
    out = kernel(eps, dts_a)
    print("ran ok", out.shape, out.dtype)



# revision 39
# speedup vs baseline: 1.0416x; 1.0415x over previous
"""Maxwell rheological model kernel for Trainium2 (8 NeuronCores, SPMD).

Recurrence per batch row (a = E/ETA = 2, E_INFTY = 1, E = 2):
    gamma[0] = 0
    gamma[n+1] = (1 - 2*dt[n]) * gamma[n] + 2*dt[n] * eps[n]
    sigma[n+1] = 3*eps[n+1] - 2*gamma[n+1];  sigma[0] = 0

fp16 wire format (inputs quantized to fp16 on host, output returned as
fp16 and scaled back on host): halves HBM traffic vs f32, which is the
bottleneck for this memory-bound problem. Verified max rel err 1.6e-2
vs the f32 reference (gate 2e-2), dominated by input quantization.

On-chip math per core (all fp16 tiles, scan carry is fp32 internal):
    C[m] = 1 - 2*dt[m]     (ACT; C=0 at row starts -> scan resets)
    D[m] = dt[m]*eps[m]    (DVE tensor_tensor, 2x mode)
    Z    = scan(C, D)      (DVE tensor_tensor_scan, 2.0 cyc/elem)
    E75s[m] = 0.75*eps[m+1](ACT, absorbs the odd offset)
    S[m] = E75s[m] - Z[m]  (DVE tensor_tensor, 2x mode) = sigma[m+1]/4
Host multiplies by 4 and zeroes column 0.

Pipeline notes (the difference between 158us and 127us):
  - Loads are HWDGE on the SP queue; stores are SWDGE (nc.gpsimd) from
    the otherwise-idle Pool queue. HWDGE stores would share the 8 DMAHW
    completion lanes with the loads, and a load dispatch waits on the
    previous DMA of its lane, so loads inherit the stores' wait-on-S
    and the pipeline serializes at ~18.5us/chunk instead of 13us.
  - Keep D on the DVE. Offloading D to GPSIMD loses ~25%: the Pool op
    (8.9us vs 2.2us on DVE) waits on load completions and lands on the
    critical path through the tile scheduler's buffer-WAR sem encodings.
  - S on GPSIMD works ONLY for spread-out chunks (1,3,5,6) with a
    deep dedicated e75 pool: S inputs are produced on-chip (no load
    coupling), and alternating keeps the Pool queue under the chunk
    period. 6-of-8 chunks on Pool regresses ~9%; all 8 regresses ~30%.
  - Row-start C zeroing runs on ACT (scale=0 copy): a DVE memset can
    enter a 2-port perf mode and lock GPSIMD off the shared SBUF port
    while SWDGE store descriptors need writing.
Steady state: DVE is 100% busy at 12.98us/chunk = D 2.18 + S 2.20 +
scan 8.60 (pure exec, no gaps); ACT 7.5us and DMA ~7us fit underneath.

Layout trick: the per-core [2048, 2048] shard is viewed as [128, 16*2048]
(partition p owns 16 consecutive DRAM rows), so every DMA moves
contiguous multi-KB lines per partition. Rows are concatenated along the
free dim; the scan crosses row boundaries but C=0 there resets the
recurrence exactly. First/last chunks are processed in column segments
(chained scans) to shorten the pipeline head and tail.

Batch is sharded across 8 cores (data parallel, no collectives).
"""

import os
import sys

# Reset NeuronCores on open: after many NEFF loads in one boot the device
# drifts into a ~20-40% slower execution state; a core reset at runtime
# init restores nominal engine clocks. Set before the runtime loads.
os.environ.setdefault("NEURON_RT_RESET_CORES", "1")

if "/opt/trn_rl_repo" not in sys.path:
    sys.path.insert(0, "/opt/trn_rl_repo")

import numpy as np

import concourse.bacc as bacc
import concourse.mybir as mybir
from concourse.bass_utils import run_bass_kernel_spmd
from concourse.tile import TileContext

B, T = 16384, 2048
N_CORES = 8
B_CORE = B // N_CORES          # 2048 rows per core
P = 128
ROWS_PER_PART = B_CORE // P    # 16 DRAM rows per partition
R = 2                          # rows per chunk
N = R * T                      # free-dim elements per chunk
N_CHUNKS = ROWS_PER_PART // R
W = ROWS_PER_PART * T          # 32768 free-dim elements per partition

_prog = None


def _build():
    f16 = mybir.dt.float16
    Alu = mybir.AluOpType
    Act = mybir.ActivationFunctionType
    nc = bacc.Bacc(
        "TRN2",
        target_bir_lowering=False,
        debug=False,
        enable_asserts=False,
    )
    strains = nc.dram_tensor("strains", [P, W], f16, kind="ExternalInput").ap()
    dts = nc.dram_tensor("dts", [P, W], f16, kind="ExternalInput").ap()
    out = nc.dram_tensor("out", [P, W], f16, kind="ExternalOutput").ap()
    # All compute on DVE: offloading D to GPSIMD was tried and lost ~10%%
    # end to end -- the Pool D (8.9us exec + load-completion wait) exceeds
    # the DVE's own 2x-mode D cost inside the 13us chunk budget, because
    # load prefetch is bounded by the scheduler's buffer-WAR encodings.
    GP_D_CHUNKS = ()
    # S runs on GPSIMD for alternate middle chunks: halves the DVE's S load
    # (the DVE keeps only scan+D on those chunks) while the Pool S (8.9us)
    # hides inside the 11.9us chunk period. S inputs are produced on-chip,
    # so unlike D there is no load-completion coupling.
    GP_S_CHUNKS = (1, 3, 5)
    with TileContext(nc) as tc:
        with (
            tc.tile_pool(name="pin", bufs=6) as pin,
            tc.tile_pool(name="pc", bufs=2) as pc,
            tc.tile_pool(name="pe", bufs=3) as pe,
            tc.tile_pool(name="pd", bufs=3) as pd,
            tc.tile_pool(name="pz", bufs=2) as pz,
            tc.tile_pool(name="pout", bufs=3) as pout,
        ):
            # Warm up the Pool tensor_tensor ucode (IRAM load) while the
            # first DMAs are in flight.
            warm = pd.tile([P, 8], f16, tag="warm")
            nc.gpsimd.memset(warm[:, :], 0.0)
            nc.gpsimd.tensor_tensor(
                out=warm[:, :], in0=warm[:, :], in1=warm[:, :], op=Alu.mult
            )

            # Stores are emitted one chunk late: a store dispatch waits on
            # its S, and the ACT sequencer (which dispatches stores) would
            # otherwise head-of-line block the next chunk's C/zeros/e75
            # behind that wait, serializing the whole pipeline.
            pending_store = []

            def flush_stores():
                # SWDGE (Pool) stores: HWDGE stores would share the 8 DMAHW
                # completion lanes with the loads, and a load dispatch waits
                # on the previous DMA of its lane -- loads end up inheriting
                # the stores' wait-on-S and the pipeline serializes.
                for args in pending_store:
                    nc.gpsimd.dma_start(out=args[0], in_=args[1])
                pending_store.clear()

            for k in range(N_CHUNKS):
                base = k * N
                dt_t = pin.tile([P, N], f16, tag="dt")
                ep_t = pin.tile([P, N], f16, tag="eps")
                c_t = pc.tile([P, N], f16, tag="c")
                # e75 gets its own deep pool: gp-S chunks read it on the Pool
                # engine, and a shallow shared pool would make ACT's next e75
                # WAR-wait on that slow read, stalling the ACT queue.
                e_t = pe.tile([P, N], f16, tag="e75")
                d_t = pd.tile([P, N], f16, tag="d")
                z_t = pz.tile([P, N], f16, tag="z")
                s_t = pout.tile([P, N], f16, tag="sig")

                # First chunk: segment the loads + compute (chained scans) so
                # the DVE starts ~10us earlier. T-sized segments align with
                # row boundaries, so every segment scan starts with C=0 and a
                # 0.0 initial is exact.
                if k == 0:
                    # Fine head segments: the DVE starts once 512 columns
                    # land; sub-row segments chain the previous z as initial.
                    bounds = [0, 512, 1024, T, N]
                elif k == N_CHUNKS - 1:
                    # Finer tail: the last scan/S/store drain only 1024 cols.
                    bounds = [0, T, T + 1024, N]
                else:
                    bounds = [0, N]
                segs = list(zip(bounds[:-1], bounds[1:]))

                for lo, hi in segs:
                    nc.sync.dma_start(
                        out=dt_t[:, lo:hi], in_=dts[:, base + lo : base + hi]
                    )
                    nc.sync.dma_start(
                        out=ep_t[:, lo:hi], in_=strains[:, base + lo : base + hi]
                    )

                for lo, hi in segs:
                    # C[m] = 1 - 2*dt[m]; C = 0 at row starts (gamma[0] = 0).
                    # Zeros via ACT (scale=0): a DVE memset could enter a
                    # 2-port perf mode and lock GPSIMD off the shared port.
                    # Sub-row segments (lo % T != 0) compute C from lo and
                    # may contain no row start.
                    clo = lo + 1 if lo % T == 0 else lo
                    nc.scalar.activation(
                        out=c_t[:, clo:hi],
                        in_=dt_t[:, clo:hi],
                        func=Act.Copy,
                        scale=-2.0,
                        bias=1.0,
                    )
                    rs0 = -(-lo // T) * T
                    if rs0 < hi:
                        nc.scalar.activation(
                            out=c_t[:, rs0:hi:T],
                            in_=dt_t[:, rs0:hi:T],
                            func=Act.Copy,
                            scale=0.0,
                        )

                    # D[m] = dt[m]*eps[m] (scan data1; the row-start value is
                    # the reset gamma_1/2 = dt0*eps0).
                    d_engine = nc.gpsimd if k in GP_D_CHUNKS else nc.vector
                    d_engine.tensor_tensor(
                        out=d_t[:, lo:hi],
                        in0=dt_t[:, lo:hi],
                        in1=ep_t[:, lo:hi],
                        op=Alu.mult,
                    )


                for lo, hi in segs:
                    # E75s[m] = 0.75*eps[m+1] (ACT absorbs the +1 shift so the
                    # subtract stays 4B-aligned for the DVE 2x mode).
                    ch = min(hi, N - 1)
                    nc.scalar.activation(
                        out=e_t[:, lo:ch],
                        in_=ep_t[:, lo + 1 : ch + 1],
                        func=Act.Copy,
                        scale=0.75,
                    )

                    # Z = scan(C, D): z[m] = C[m]*z[m-1] + D[m] (fp32 carry).
                    # Segments start at row boundaries: initial is moot.
                    nc.vector.tensor_tensor_scan(
                        out=z_t[:, lo:hi],
                        data0=c_t[:, lo:hi],
                        data1=d_t[:, lo:hi],
                        initial=0.0 if lo % T == 0 else z_t[:, lo - 1 : lo],
                        op0=Alu.mult,
                        op1=Alu.add,
                    )

                    # S[m] = E75s[m] - Z[m] = sigma[m+1]/4
                    s_eng = nc.gpsimd if k in GP_S_CHUNKS else nc.vector
                    s_eng.tensor_tensor(
                        out=s_t[:, lo:ch],
                        in0=e_t[:, lo:ch],
                        in1=z_t[:, lo:ch],
                        op=Alu.subtract,
                    )

                    # Store sigma[base+lo+1 : base+ch+1]; column-0 positions
                    # of each row are never stored (host writes sigma[0]=0).
                    # SWDGE store from the otherwise-idle Pool queue: it never
                    # shares DMAHW completion lanes with the loads and nothing
                    # queues behind its wait-on-S.
                    nc.gpsimd.dma_start(
                        out=out[:, base + lo + 1 : base + ch + 1],
                        in_=s_t[:, lo:ch],
                    )
    nc.compile()
    return nc


def _get_prog():
    global _prog
    if _prog is None:
        _prog = _build()
    return _prog


def _run(strains, dts, **kwargs):
    nc = _get_prog()
    s16 = np.ascontiguousarray(strains, dtype=np.float16).reshape(
        N_CORES, P, W
    )
    d16 = np.ascontiguousarray(dts, dtype=np.float16).reshape(N_CORES, P, W)
    in_maps = [
        {"strains": s16[c], "dts": d16[c]} for c in range(N_CORES)
    ]
    res = run_bass_kernel_spmd(nc, in_maps, core_ids=list(range(N_CORES)), **kwargs)
    parts = [np.asarray(r["out"]).reshape(B_CORE, T) for r in res.results]
    full = np.concatenate(parts, axis=0).astype(np.float32)
    full *= 4.0
    full[:, 0] = 0.0
    return full, res


def kernel(strains, dts):
    out, _ = _run(strains, dts)
    return out


if __name__ == "__main__":
    rng = np.random.default_rng(0)
    eps = rng.standard_normal((B, T), dtype=np.float32)
    dts_a = rng.random((B, T), dtype=np.float32)
    out = kernel(eps, dts_a)
    print("ran ok", out.shape, out.dtype)



# revision 41
# speedup vs baseline: 1.0464x; 1.0046x over previous
"""Maxwell rheological model kernel for Trainium2 (8 NeuronCores, SPMD).

Recurrence per batch row (a = E/ETA = 2, E_INFTY = 1, E = 2):
    gamma[0] = 0
    gamma[n+1] = (1 - 2*dt[n]) * gamma[n] + 2*dt[n] * eps[n]
    sigma[n+1] = 3*eps[n+1] - 2*gamma[n+1];  sigma[0] = 0

fp16 wire format (inputs quantized to fp16 on host, output returned as
fp16 and scaled back on host): halves HBM traffic vs f32, which is the
bottleneck for this memory-bound problem. Verified max rel err 1.6e-2
vs the f32 reference (gate 2e-2), dominated by input quantization.

On-chip math per core (all fp16 tiles, scan carry is fp32 internal):
    C[m] = 1 - 2*dt[m]     (ACT; C=0 at row starts -> scan resets)
    D[m] = dt[m]*eps[m]    (DVE tensor_tensor, 2x mode)
    Z    = scan(C, D)      (DVE tensor_tensor_scan, 2.0 cyc/elem)
    E75s[m] = 0.75*eps[m+1](ACT, absorbs the odd offset)
    S[m] = E75s[m] - Z[m]  (DVE tensor_tensor, 2x mode) = sigma[m+1]/4
Host multiplies by 4 and zeroes column 0.

Pipeline notes (the difference between 158us and 127us):
  - Loads are HWDGE on the SP queue; stores are SWDGE (nc.gpsimd) from
    the otherwise-idle Pool queue. HWDGE stores would share the 8 DMAHW
    completion lanes with the loads, and a load dispatch waits on the
    previous DMA of its lane, so loads inherit the stores' wait-on-S
    and the pipeline serializes at ~18.5us/chunk instead of 13us.
  - Keep D on the DVE. Offloading D to GPSIMD loses ~25%: the Pool op
    (8.9us vs 2.2us on DVE) waits on load completions and lands on the
    critical path through the tile scheduler's buffer-WAR sem encodings.
  - S on GPSIMD works ONLY for strictly alternating chunks (1,3,5)
    with a dedicated e75 pool: S inputs are produced on-chip (no load
    coupling). Consecutive Pool chunks retire S late, and load WAR
    waits merged onto the Pool clock inherit that lateness ((1,3,5,6)
    costs +6us; 6-of-8 chunks +9%; all 8 +30%).
  - Row-start C zeroing runs on ACT (scale=0 copy): a DVE memset can
    enter a 2-port perf mode and lock GPSIMD off the shared SBUF port
    while SWDGE store descriptors need writing.
Steady state: DVE is 100% busy at 12.98us/chunk = D 2.18 + S 2.20 +
scan 8.60 (pure exec, no gaps); ACT 7.5us and DMA ~7us fit underneath.

Layout trick: the per-core [2048, 2048] shard is viewed as [128, 16*2048]
(partition p owns 16 consecutive DRAM rows), so every DMA moves
contiguous multi-KB lines per partition. Rows are concatenated along the
free dim; the scan crosses row boundaries but C=0 there resets the
recurrence exactly. First/last chunks are processed in column segments
(chained scans) to shorten the pipeline head and tail.

Batch is sharded across 8 cores (data parallel, no collectives).
"""

import os
import sys

# Reset NeuronCores on open: after many NEFF loads in one boot the device
# drifts into a ~20-40% slower execution state; a core reset at runtime
# init restores nominal engine clocks. Set before the runtime loads.
os.environ.setdefault("NEURON_RT_RESET_CORES", "1")

if "/opt/trn_rl_repo" not in sys.path:
    sys.path.insert(0, "/opt/trn_rl_repo")

import numpy as np

import concourse.bacc as bacc
import concourse.mybir as mybir
from concourse.bass_utils import run_bass_kernel_spmd
from concourse.tile import TileContext

B, T = 16384, 2048
N_CORES = 8
B_CORE = B // N_CORES          # 2048 rows per core
P = 128
ROWS_PER_PART = B_CORE // P    # 16 DRAM rows per partition
R = 2                          # rows per chunk
N = R * T                      # free-dim elements per chunk
N_CHUNKS = ROWS_PER_PART // R
W = ROWS_PER_PART * T          # 32768 free-dim elements per partition

_prog = None


def _build():
    f16 = mybir.dt.float16
    Alu = mybir.AluOpType
    Act = mybir.ActivationFunctionType
    nc = bacc.Bacc(
        "TRN2",
        target_bir_lowering=False,
        debug=False,
        enable_asserts=False,
    )
    strains = nc.dram_tensor("strains", [P, W], f16, kind="ExternalInput").ap()
    dts = nc.dram_tensor("dts", [P, W], f16, kind="ExternalInput").ap()
    out = nc.dram_tensor("out", [P, W], f16, kind="ExternalOutput").ap()
    # All compute on DVE: offloading D to GPSIMD was tried and lost ~10%%
    # end to end -- the Pool D (8.9us exec + load-completion wait) exceeds
    # the DVE's own 2x-mode D cost inside the 13us chunk budget, because
    # load prefetch is bounded by the scheduler's buffer-WAR encodings.
    GP_D_CHUNKS = ()
    # S runs on GPSIMD for alternate middle chunks: halves the DVE's S load
    # (the DVE keeps only scan+D on those chunks) while the Pool S (8.9us)
    # hides inside the 11.9us chunk period. S inputs are produced on-chip,
    # so unlike D there is no load-completion coupling.
    GP_S_CHUNKS = (1, 3, 5)
    with TileContext(nc) as tc:
        with (
            tc.tile_pool(name="pin", bufs=5) as pin,
            tc.tile_pool(name="pc", bufs=2) as pc,
            tc.tile_pool(name="pe", bufs=4) as pe,
            tc.tile_pool(name="pd", bufs=3) as pd,
            tc.tile_pool(name="pz", bufs=3) as pz,
            tc.tile_pool(name="pout", bufs=3) as pout,
        ):
            # Warm up the Pool tensor_tensor ucode (IRAM load) while the
            # first DMAs are in flight.
            warm = pd.tile([P, 8], f16, tag="warm")
            nc.gpsimd.memset(warm[:, :], 0.0)
            nc.gpsimd.tensor_tensor(
                out=warm[:, :], in0=warm[:, :], in1=warm[:, :], op=Alu.mult
            )

            # Stores are emitted one chunk late: a store dispatch waits on
            # its S, and the ACT sequencer (which dispatches stores) would
            # otherwise head-of-line block the next chunk's C/zeros/e75
            # behind that wait, serializing the whole pipeline.
            pending_store = []

            def flush_stores():
                # SWDGE (Pool) stores: HWDGE stores would share the 8 DMAHW
                # completion lanes with the loads, and a load dispatch waits
                # on the previous DMA of its lane -- loads end up inheriting
                # the stores' wait-on-S and the pipeline serializes.
                for args in pending_store:
                    nc.gpsimd.dma_start(out=args[0], in_=args[1])
                pending_store.clear()

            for k in range(N_CHUNKS):
                base = k * N
                dt_t = pin.tile([P, N], f16, tag="dt")
                ep_t = pin.tile([P, N], f16, tag="eps")
                c_t = pc.tile([P, N], f16, tag="c")
                # e75 gets its own deep pool: gp-S chunks read it on the Pool
                # engine, and a shallow shared pool would make ACT's next e75
                # WAR-wait on that slow read, stalling the ACT queue.
                e_t = pe.tile([P, N], f16, tag="e75")
                d_t = pd.tile([P, N], f16, tag="d")
                z_t = pz.tile([P, N], f16, tag="z")
                s_t = pout.tile([P, N], f16, tag="sig")

                # First chunk: segment the loads + compute (chained scans) so
                # the DVE starts ~10us earlier. T-sized segments align with
                # row boundaries, so every segment scan starts with C=0 and a
                # 0.0 initial is exact.
                if k == 0:
                    # Fine head segments: the DVE starts once 512 columns
                    # land; sub-row segments chain the previous z as initial.
                    bounds = [0, 512, 1024, T, N]
                elif k == N_CHUNKS - 1:
                    # Finer tail: the last scan/S/store drain only 1024 cols.
                    bounds = [0, T, T + 1024, N]
                else:
                    bounds = [0, N]
                segs = list(zip(bounds[:-1], bounds[1:]))

                for lo, hi in segs:
                    nc.sync.dma_start(
                        out=dt_t[:, lo:hi], in_=dts[:, base + lo : base + hi]
                    )
                    nc.sync.dma_start(
                        out=ep_t[:, lo:hi], in_=strains[:, base + lo : base + hi]
                    )

                for lo, hi in segs:
                    # C[m] = 1 - 2*dt[m]; C = 0 at row starts (gamma[0] = 0).
                    # Zeros via ACT (scale=0): a DVE memset could enter a
                    # 2-port perf mode and lock GPSIMD off the shared port.
                    # Sub-row segments (lo % T != 0) compute C from lo and
                    # may contain no row start.
                    clo = lo + 1 if lo % T == 0 else lo
                    nc.scalar.activation(
                        out=c_t[:, clo:hi],
                        in_=dt_t[:, clo:hi],
                        func=Act.Copy,
                        scale=-2.0,
                        bias=1.0,
                    )
                    rs0 = -(-lo // T) * T
                    if rs0 < hi:
                        nc.scalar.activation(
                            out=c_t[:, rs0:hi:T],
                            in_=dt_t[:, rs0:hi:T],
                            func=Act.Copy,
                            scale=0.0,
                        )

                    # D[m] = dt[m]*eps[m] (scan data1; the row-start value is
                    # the reset gamma_1/2 = dt0*eps0).
                    d_engine = nc.gpsimd if k in GP_D_CHUNKS else nc.vector
                    d_engine.tensor_tensor(
                        out=d_t[:, lo:hi],
                        in0=dt_t[:, lo:hi],
                        in1=ep_t[:, lo:hi],
                        op=Alu.mult,
                    )


                for lo, hi in segs:
                    # E75s[m] = 0.75*eps[m+1] (ACT absorbs the +1 shift so the
                    # subtract stays 4B-aligned for the DVE 2x mode).
                    ch = min(hi, N - 1)
                    nc.scalar.activation(
                        out=e_t[:, lo:ch],
                        in_=ep_t[:, lo + 1 : ch + 1],
                        func=Act.Copy,
                        scale=0.75,
                    )

                    # Z = scan(C, D): z[m] = C[m]*z[m-1] + D[m] (fp32 carry).
                    # Segments start at row boundaries: initial is moot.
                    nc.vector.tensor_tensor_scan(
                        out=z_t[:, lo:hi],
                        data0=c_t[:, lo:hi],
                        data1=d_t[:, lo:hi],
                        initial=0.0 if lo % T == 0 else z_t[:, lo - 1 : lo],
                        op0=Alu.mult,
                        op1=Alu.add,
                    )

                    # S[m] = E75s[m] - Z[m] = sigma[m+1]/4
                    s_eng = nc.gpsimd if k in GP_S_CHUNKS else nc.vector
                    s_eng.tensor_tensor(
                        out=s_t[:, lo:ch],
                        in0=e_t[:, lo:ch],
                        in1=z_t[:, lo:ch],
                        op=Alu.subtract,
                    )

                    # Store sigma[base+lo+1 : base+ch+1]; column-0 positions
                    # of each row are never stored (host writes sigma[0]=0).
                    # SWDGE store from the otherwise-idle Pool queue: it never
                    # shares DMAHW completion lanes with the loads and nothing
                    # queues behind its wait-on-S.
                    nc.gpsimd.dma_start(
                        out=out[:, base + lo + 1 : base + ch + 1],
                        in_=s_t[:, lo:ch],
                    )
    nc.compile()
    return nc


def _get_prog():
    global _prog
    if _prog is None:
        _prog = _build()
    return _prog


def _run(strains, dts, **kwargs):
    nc = _get_prog()
    s16 = np.ascontiguousarray(strains, dtype=np.float16).reshape(
        N_CORES, P, W
    )
    d16 = np.ascontiguousarray(dts, dtype=np.float16).reshape(N_CORES, P, W)
    in_maps = [
        {"strains": s16[c], "dts": d16[c]} for c in range(N_CORES)
    ]
    res = run_bass_kernel_spmd(nc, in_maps, core_ids=list(range(N_CORES)), **kwargs)
    parts = [np.asarray(r["out"]).reshape(B_CORE, T) for r in res.results]
    full = np.concatenate(parts, axis=0).astype(np.float32)
    full *= 4.0
    full[:, 0] = 0.0
    return full, res


def kernel(strains, dts):
    out, _ = _run(strains, dts)
    return out


if __name__ == "__main__":
    rng = np.random.default_rng(0)
    eps = rng.standard_normal((B, T), dtype=np.float32)
    dts_a = rng.random((B, T), dtype=np.float32)
    out = kernel(eps, dts_a)
    print("ran ok", out.shape, out.dtype)

